# revision 10
# baseline (speedup 1.0000x reference)
"""Trainium2 Bass kernel for nn_CACProjector (logits = x @ W^T, CAC distances).

Strategy: data-parallel over batch B across 8 NeuronCores. Each core gets a
(768, 2048) column-slice xT of x^T (host-side transpose so the contraction
dim D lands on SBUF partitions) and a replicated W^T (768, 1024). On-core:

  logits[b, c] = sum_d xT[d, b] * wT[d, c]      (PE, fp32 accumulate in PSUM)
  sq_norm[b]   = sum_c logits[b, c]^2           (ACT Square pass w/ accum_out,
                                                 reads fp32 PSUM)
  dist[b, c]   = sqrt(sq_norm[b] + alpha^2 - 2*alpha*logits[b, c])
                                                 (ACT Sqrt w/ scale+bias,
                                                 reads fp32 PSUM)

d2 = ||l||^2 - 2a*l_j + a^2 >= (l_j - a)^2 >= 0 mathematically, and with this
data d2 ~ 1100 >> 0, so the reference's maximum(d2, 0) clamp is a no-op.

The kernel is HBM-bandwidth bound, so I/O transport precision is the main
lever. IO_MODE picks it:
  "bf16": x/W shipped bf16, logits/dist stored bf16 (fp32 PSUM accumulate and
          fp32 distance math throughout) -> ~13 MB/core of HBM traffic.
  "f32r": x/W shipped fp32 (TF32-rounded for full-rate PE), outputs fp32
          -> ~26 MB/core, rel err ~3e-4.
"""

import sys

sys.path.insert(0, "/opt/trn_rl_repo")

from contextlib import ExitStack

import ml_dtypes
import numpy as np

import concourse.tile as tile
from concourse import bacc, mybir
from concourse.bass_utils import run_bass_kernel_spmd

N_CORES = 8
B, D, C = 16384, 768, 1024
BS = B // N_CORES          # 2048 rows of B per core
P = 128                    # partition dim
KT = D // P                # 6 contraction chunks
NBT = BS // P              # 16 output row-tiles per core
ALPHA = 10.0

F32 = mybir.dt.float32
F32R = mybir.dt.float32r
BF16 = mybir.dt.bfloat16

IO_MODE = "bf16"


def build(io_mode=IO_MODE):
    in_dt = BF16 if io_mode == "bf16" else F32R
    out_dt = BF16 if io_mode == "bf16" else F32

    nc = bacc.Bacc("TRN2", target_bir_lowering=False, debug=False)
    xT = nc.dram_tensor("xT", [D, BS], in_dt, kind="ExternalInput").ap()
    wT = nc.dram_tensor("wT", [D, C], in_dt, kind="ExternalInput").ap()
    logits = nc.dram_tensor("logits", [BS, C], out_dt, kind="ExternalOutput").ap()
    dist = nc.dram_tensor("dist", [BS, C], out_dt, kind="ExternalOutput").ap()

    HB = BS // 2  # x tiles DMA'd in two free-dim halves for a faster ramp-in

    with tile.TileContext(nc) as tc, ExitStack() as ctx:
        xpool = ctx.enter_context(tc.tile_pool(name="xT", bufs=1))
        wpool = ctx.enter_context(tc.tile_pool(name="wT", bufs=1))
        psum = ctx.enter_context(tc.tile_pool(name="psum", bufs=4, space="PSUM"))
        lpool = ctx.enter_context(tc.tile_pool(name="lg", bufs=3))
        dpool = ctx.enter_context(tc.tile_pool(name="dist", bufs=3))
        spool = ctx.enter_context(tc.tile_pool(name="sq", bufs=2))
        npool = ctx.enter_context(tc.tile_pool(name="norms", bufs=3))

        # Ramp-in: the very first matmul needs only x[d0:128, b0:512] and
        # wT[d0:128, c0:512], so k=0 gets quarter-size x tiles and a split w
        # tile, emitted first; bulk tiles follow. Emission order == HWDGE
        # FIFO service order, so early b-tiles start while the rest streams.
        xtiles = {}   # (k, half) -> (tile, col0) list
        wt_lo, wt_hi = [], []

        # w loads ride the Scalar HWDGE ring so their ~0.6 us issue slots run
        # in parallel with the x loads on the Sync ring.
        x0q = []
        for q in range(2):
            t = xpool.tile([P, 512], in_dt, tag=f"x0q{q}")
            nc.sync.dma_start(t[:], xT[0:P, q * 512 : (q + 1) * 512])
            x0q.append(t)
            if q == 0:
                w0lo = wpool.tile([P, 512], in_dt, tag="w0lo")
                nc.scalar.dma_start(w0lo[:], wT[0:P, 0:512])
                w0hi = wpool.tile([P, 512], in_dt, tag="w0hi")
                nc.scalar.dma_start(w0hi[:], wT[0:P, 512:1024])
                wt_lo.append(w0lo)
                wt_hi.append(w0hi)
        xtiles[(0, 0)] = [(x0q[0], 0), (x0q[1], 512)]

        for k in range(1, KT):
            xka = xpool.tile([P, HB], in_dt, tag=f"xa{k}")
            nc.sync.dma_start(xka[:], xT[k * P : (k + 1) * P, :HB])
            xtiles[(k, 0)] = [(xka, 0)]
            wk = wpool.tile([P, C], in_dt, tag=f"w{k}")
            nc.scalar.dma_start(wk[:], wT[k * P : (k + 1) * P, :])
            wt_lo.append(wk[:, 0:512])
            wt_hi.append(wk[:, 512:1024])
        for k in range(KT):
            xkb = xpool.tile([P, HB], in_dt, tag=f"xb{k}")
            nc.sync.dma_start(xkb[:], xT[k * P : (k + 1) * P, HB:])
            xtiles[(k, 1)] = [(xkb, 0)]

        def x_slice(k, bt):
            half, boff = divmod(bt * P, HB)
            for t, col0 in reversed(xtiles[(k, half)]):
                if boff >= col0:
                    return t[:, boff - col0 : boff - col0 + P]
            raise AssertionError

        for bt in range(NBT):
            ps = psum.tile([P, C], F32)
            for k in range(KT):
                lhs = x_slice(k, bt)
                nc.tensor.matmul(
                    ps[:, 0:512], lhs, wt_lo[k], start=(k == 0), stop=(k == KT - 1)
                )
                nc.tensor.matmul(
                    ps[:, 512:1024], lhs, wt_hi[k], start=(k == 0), stop=(k == KT - 1)
                )

            # PSUM -> SBUF bf16 copy split across DVE (half 0) and ACT
            # (half 1): releases PSUM in ~0.7 us and keeps ACT's queue free
            # of anything the Sqrt's DVE-side deps could head-of-line block.
            lg = lpool.tile([P, C], out_dt)
            nc.vector.tensor_copy(lg[:, 0:512], ps[:, 0:512])
            nc.scalar.copy(lg[:, 512:1024], ps[:, 512:1024])

            sq = spool.tile([P, C], out_dt)
            nc.vector.tensor_tensor(sq[:], lg[:], lg[:], mybir.AluOpType.mult)
            sn = npool.tile([P, 1], F32, tag="sn")
            nc.vector.tensor_reduce(
                sn[:], sq[:], axis=mybir.AxisListType.X, op=mybir.AluOpType.add
            )
            snb = npool.tile([P, 1], F32, tag="snb")
            nc.vector.tensor_scalar_add(snb[:], sn[:], ALPHA * ALPHA)

            dt_ = dpool.tile([P, C], out_dt)
            nc.scalar.activation(
                dt_[:],
                lg[:],
                mybir.ActivationFunctionType.Sqrt,
                bias=snb[:],
                scale=-2.0 * ALPHA,
            )

            nc.sync.dma_start(logits[bt * P : (bt + 1) * P, :], lg[:])
            nc.sync.dma_start(dist[bt * P : (bt + 1) * P, :], dt_[:])

    nc.compile()
    return nc


_NC = {}


def _round_tf32(a):
    """Round-to-nearest-even to TF32 (10-bit mantissa) in fp32 storage.

    The FP32r PE mode multiplies at TF32 precision and the BIR contract is
    that f32r operands arrive pre-rounded; carry into the exponent on
    mantissa overflow is exactly what RNE needs (inf/nan inputs don't occur
    here).
    """
    u = a.view(np.uint32)
    r = (u + np.uint32(0xFFF) + ((u >> np.uint32(13)) & np.uint32(1))) & np.uint32(
        0xFFFFE000
    )
    return r.view(np.float32)


def kernel(x, W, trace=False, _result_box=None, io_mode=IO_MODE):
    if io_mode not in _NC:
        _NC[io_mode] = build(io_mode)
    nc = _NC[io_mode]

    x = np.ascontiguousarray(np.asarray(x, dtype=np.float32))
    W = np.ascontiguousarray(np.asarray(W, dtype=np.float32))
    if io_mode == "bf16":
        prep = lambda a: np.asarray(a, dtype=ml_dtypes.bfloat16)
    else:
        prep = _round_tf32
    wT = prep(np.ascontiguousarray(W.T))
    in_maps = [
        {
            "xT": prep(np.ascontiguousarray(x[i * BS : (i + 1) * BS, :].T)),
            "wT": wT,
        }
        for i in range(N_CORES)
    ]

    res = run_bass_kernel_spmd(nc, in_maps, list(range(N_CORES)), trace=trace)
    if _result_box is not None:
        _result_box.append(res)

    logits = np.concatenate(
        [np.asarray(res.results[i]["logits"], dtype=np.float32) for i in range(N_CORES)],
        axis=0,
    )
    dist = np.concatenate(
        [np.asarray(res.results[i]["dist"], dtype=np.float32) for i in range(N_CORES)],
        axis=0,
    )
    return logits, dist


# revision 12
# speedup vs baseline: 1.0405x; 1.0405x over previous
"""Trainium2 Bass kernel for nn_CACProjector (logits = x @ W^T, CAC distances).

Strategy: data-parallel over batch B across 8 NeuronCores. Each core gets a
(768, 2048) column-slice xT of x^T (host-side transpose so the contraction
dim D lands on SBUF partitions) and a replicated W^T (768, 1024). On-core:

  logits[b, c] = sum_d xT[d, b] * wT[d, c]      (PE, fp32 accumulate in PSUM)
  sq_norm[b]   = sum_c logits[b, c]^2           (ACT Square pass w/ accum_out,
                                                 reads fp32 PSUM)
  dist[b, c]   = sqrt(sq_norm[b] + alpha^2 - 2*alpha*logits[b, c])
                                                 (ACT Sqrt w/ scale+bias,
                                                 reads fp32 PSUM)

d2 = ||l||^2 - 2a*l_j + a^2 >= (l_j - a)^2 >= 0 mathematically, and with this
data d2 ~ 1100 >> 0, so the reference's maximum(d2, 0) clamp is a no-op.

The kernel is HBM-bandwidth bound, so I/O transport precision is the main
lever. IO_MODE picks it:
  "bf16": x/W shipped bf16, logits/dist stored bf16 (fp32 PSUM accumulate and
          fp32 distance math throughout) -> ~13 MB/core of HBM traffic.
  "f32r": x/W shipped fp32 (TF32-rounded for full-rate PE), outputs fp32
          -> ~26 MB/core, rel err ~3e-4.
"""

import sys

sys.path.insert(0, "/opt/trn_rl_repo")

from contextlib import ExitStack

import ml_dtypes
import numpy as np

import concourse.tile as tile
from concourse import bacc, mybir
from concourse.bass_utils import run_bass_kernel_spmd

N_CORES = 8
B, D, C = 16384, 768, 1024
BS = B // N_CORES          # 2048 rows of B per core
P = 128                    # partition dim
KT = D // P                # 6 contraction chunks
NBT = BS // P              # 16 output row-tiles per core
ALPHA = 10.0

F32 = mybir.dt.float32
F32R = mybir.dt.float32r
BF16 = mybir.dt.bfloat16

IO_MODE = "bf16"


def build(io_mode=IO_MODE):
    in_dt = BF16 if io_mode == "bf16" else F32R
    out_dt = BF16 if io_mode == "bf16" else F32

    nc = bacc.Bacc("TRN2", target_bir_lowering=False, debug=False)
    xT = nc.dram_tensor("xT", [D, BS], in_dt, kind="ExternalInput").ap()
    wT = nc.dram_tensor("wT", [D, C], in_dt, kind="ExternalInput").ap()
    logits = nc.dram_tensor("logits", [BS, C], out_dt, kind="ExternalOutput").ap()
    dist = nc.dram_tensor("dist", [BS, C], out_dt, kind="ExternalOutput").ap()

    HB = BS // 2  # x tiles DMA'd in two free-dim halves for a faster ramp-in

    with tile.TileContext(nc) as tc, ExitStack() as ctx:
        xpool = ctx.enter_context(tc.tile_pool(name="xT", bufs=1))
        wpool = ctx.enter_context(tc.tile_pool(name="wT", bufs=1))
        psum = ctx.enter_context(tc.tile_pool(name="psum", bufs=4, space="PSUM"))
        lpool = ctx.enter_context(tc.tile_pool(name="lg", bufs=3))
        dpool = ctx.enter_context(tc.tile_pool(name="dist", bufs=3))
        spool = ctx.enter_context(tc.tile_pool(name="sq", bufs=2))
        npool = ctx.enter_context(tc.tile_pool(name="norms", bufs=3))

        # Ramp-in: the very first matmul needs only x[d0:128, b0:512] and
        # wT[d0:128, c0:512], so k=0 gets quarter-size x tiles and a split w
        # tile, emitted first; bulk tiles follow. Emission order == HWDGE
        # FIFO service order, so early b-tiles start while the rest streams.
        xtiles = {}   # (k, half) -> (tile, col0) list
        wt_lo, wt_hi = [], []

        x0q = []
        for q in range(2):
            t = xpool.tile([P, 512], in_dt, tag=f"x0q{q}")
            nc.sync.dma_start(t[:], xT[0:P, q * 512 : (q + 1) * 512])
            x0q.append(t)
            if q == 0:
                w0lo = wpool.tile([P, 512], in_dt, tag="w0lo")
                nc.sync.dma_start(w0lo[:], wT[0:P, 0:512])
                w0hi = wpool.tile([P, 512], in_dt, tag="w0hi")
                nc.sync.dma_start(w0hi[:], wT[0:P, 512:1024])
                wt_lo.append(w0lo)
                wt_hi.append(w0hi)
        xtiles[(0, 0)] = [(x0q[0], 0), (x0q[1], 512)]

        for k in range(1, KT):
            xka = xpool.tile([P, HB], in_dt, tag=f"xa{k}")
            nc.sync.dma_start(xka[:], xT[k * P : (k + 1) * P, :HB])
            xtiles[(k, 0)] = [(xka, 0)]
            wk = wpool.tile([P, C], in_dt, tag=f"w{k}")
            nc.sync.dma_start(wk[:], wT[k * P : (k + 1) * P, :])
            wt_lo.append(wk[:, 0:512])
            wt_hi.append(wk[:, 512:1024])
        for k in range(KT):
            xkb = xpool.tile([P, HB], in_dt, tag=f"xb{k}")
            nc.sync.dma_start(xkb[:], xT[k * P : (k + 1) * P, HB:])
            xtiles[(k, 1)] = [(xkb, 0)]

        def x_slice(k, bt):
            half, boff = divmod(bt * P, HB)
            for t, col0 in reversed(xtiles[(k, half)]):
                if boff >= col0:
                    return t[:, boff - col0 : boff - col0 + P]
            raise AssertionError

        def finish(bt, lg, snb):
            dt_ = dpool.tile([P, C], out_dt)
            nc.scalar.activation(
                dt_[:],
                lg[:],
                mybir.ActivationFunctionType.Sqrt,
                bias=snb[:],
                scale=-2.0 * ALPHA,
            )
            nc.sync.dma_start(dist[bt * P : (bt + 1) * P, :], dt_[:])
            nc.sync.dma_start(logits[bt * P : (bt + 1) * P, :], lg[:])

        # The Sqrt + stores for b-tile N are emitted during b-tile N+1, after
        # the next copy: by then its bias operand (snb) has long been
        # produced, so ACT's in-order queue never idles waiting on the DVE
        # square/reduce chain.
        pending = None
        for bt in range(NBT):
            ps = psum.tile([P, C], F32)
            for k in range(KT):
                lhs = x_slice(k, bt)
                nc.tensor.matmul(
                    ps[:, 0:512], lhs, wt_lo[k], start=(k == 0), stop=(k == KT - 1)
                )
                nc.tensor.matmul(
                    ps[:, 512:1024], lhs, wt_hi[k], start=(k == 0), stop=(k == KT - 1)
                )

            # ACT is the only PSUM consumer: one Copy pass materializes bf16
            # logits and releases the PSUM banks for the next b-tile.
            lg = lpool.tile([P, C], out_dt)
            nc.scalar.copy(lg[:], ps[:])

            sq = spool.tile([P, C], out_dt)
            nc.vector.tensor_tensor(sq[:], lg[:], lg[:], mybir.AluOpType.mult)
            sn = npool.tile([P, 1], F32, tag="sn")
            nc.vector.tensor_reduce(
                sn[:], sq[:], axis=mybir.AxisListType.X, op=mybir.AluOpType.add
            )
            snb = npool.tile([P, 1], F32, tag="snb")
            nc.vector.tensor_scalar_add(snb[:], sn[:], ALPHA * ALPHA)

            if pending is not None:
                finish(*pending)
            pending = (bt, lg, snb)
        finish(*pending)

    nc.compile()
    return nc


_NC = {}


def _round_tf32(a):
    """Round-to-nearest-even to TF32 (10-bit mantissa) in fp32 storage.

    The FP32r PE mode multiplies at TF32 precision and the BIR contract is
    that f32r operands arrive pre-rounded; carry into the exponent on
    mantissa overflow is exactly what RNE needs (inf/nan inputs don't occur
    here).
    """
    u = a.view(np.uint32)
    r = (u + np.uint32(0xFFF) + ((u >> np.uint32(13)) & np.uint32(1))) & np.uint32(
        0xFFFFE000
    )
    return r.view(np.float32)


def kernel(x, W, trace=False, _result_box=None, io_mode=IO_MODE):
    if io_mode not in _NC:
        _NC[io_mode] = build(io_mode)
    nc = _NC[io_mode]

    x = np.ascontiguousarray(np.asarray(x, dtype=np.float32))
    W = np.ascontiguousarray(np.asarray(W, dtype=np.float32))
    if io_mode == "bf16":
        prep = lambda a: np.asarray(a, dtype=ml_dtypes.bfloat16)
    else:
        prep = _round_tf32
    wT = prep(np.ascontiguousarray(W.T))
    in_maps = [
        {
            "xT": prep(np.ascontiguousarray(x[i * BS : (i + 1) * BS, :].T)),
            "wT": wT,
        }
        for i in range(N_CORES)
    ]

    res = run_bass_kernel_spmd(nc, in_maps, list(range(N_CORES)), trace=trace)
    if _result_box is not None:
        _result_box.append(res)

    logits = np.concatenate(
        [np.asarray(res.results[i]["logits"], dtype=np.float32) for i in range(N_CORES)],
        axis=0,
    )
    dist = np.concatenate(
        [np.asarray(res.results[i]["dist"], dtype=np.float32) for i in range(N_CORES)],
        axis=0,
    )
    return logits, dist
